# revision 64
# baseline (speedup 1.0000x reference)
"""Banded-DTW (cosine cost, Sakoe-Chiba W=50) Trainium2 Bass kernel, v3.

Forward/backward split across 8 cores as in v2 (core c: fwd rows 0..255 of
batch group c; core c+4: same program on time-reversed inputs). Host combines
boundary rows.

v3 engine plan (per core):
- DVE runs the DP chain in fp16 (tensor_tensor min 2x_1p + tensor_tensor_scan
  4x_2p) plus the c0 sum-of-squares (TT mult + tensor_reduce) in its pre-DP
  idle window.
- Normalization is folded into the PE: each 128-row chunk is transposed via a
  matmul against diag(1/n) (built on Pool via tensor_scalar divide of an
  identity), so no elementwise normalize pass exists anywhere.
- ACT does the remaining squares (Square+accum), quad-batched sqrts, the
  PSUM->SBUF transpose evacuations (bf16, pair-packed), and the band
  evacuations (1 - ps, fp16, 6 batches per PSUM bank via partition-offset
  matmul outputs).
- Cost band staged to DRAM in fp16, diagonal-gathered back as [16, 32*101]
  tiles (same math as v2).
"""

import numpy as np
from contextlib import ExitStack

import concourse.bass as bass
import concourse.tile as tile
from concourse import mybir
from concourse.bass_utils import run_bass_kernel_spmd
from concourse.masks import make_identity

F32 = mybir.dt.float32
BF16 = mybir.dt.bfloat16
F16 = mybir.dt.float16
ALU = mybir.AluOpType
ACTF = mybir.ActivationFunctionType

NCORES = 8
B = 16         # batches per core
RH = 256       # DP rows per core (half of 512)
T = 512
DM = 256
W = 50
K = 2 * W + 1  # 101 band columns
BIG = 1.0e9    # host-side combine 'infinity'
DBIG = 30000.0  # device 'infinity' (fp16-safe)
EPS = 1e-8
X2R = 306      # x2 rows needed per half (j < 306)
RB = 64        # rows per block
NBLK = RH // RB  # 4
WBLK = 164     # staging row stride = max block j-width
STG = 64 + RB * WBLK + 64  # per-batch staging floats for one block
DP_ROWS = RH

USE_DIVIDE = True   # diag = I / n via AluOpType.divide (else reciprocal+mult)

_CACHED_NC = None


def _block_jrange(blk):
    j0 = max(0, blk * RB - W)
    j1 = blk * RB + RB - 1 + W + 1
    return j0, j1  # (0,114) (14,178) (78,242) (142,306)


def _walrus_fixup(nc, max_waits=1):
    """Walrus in this container accepts at most one sync-wait per instruction
    and rejects EVENT_SEMAPHORE_RANGE_CLEAR InstISA; hoist extra waits onto
    standalone InstEventSemaphore waits and drop the range-clear."""
    k = 0
    for bb in nc.m.functions[0].blocks:
        out = []
        for inst in bb.instructions:
            if (type(inst).__name__ == "InstISA"
                    and getattr(inst, "op_name", None) == "EVENT_SEMAPHORE_RANGE_CLEAR"):
                continue
            si = inst.sync_info
            if si is not None and len(si.on_wait) > max_waits:
                waits = list(si.on_wait)
                for w in waits[:-max_waits]:
                    es = mybir.InstEventSemaphore(
                        name=f"eswait_{k}", engine=inst.engine, ins=[], outs=[])
                    es.sync_info = mybir.SyncInfo(on_wait=[w], on_update=[])
                    out.append(es)
                    k += 1
                inst.sync_info = mybir.SyncInfo(
                    on_wait=waits[-max_waits:], on_update=list(si.on_update))
            out.append(inst)
        bb.instructions = out


def _strip_dve_serial_waits(nc):
    """Remove DVE-instruction waits on the DVE engine's own serialization
    semaphore; program order on the in-order DVE queue already sequences
    them. Cross-engine waits (DMA/ACT/Pool sems) are preserved, as are the
    updates (other engines still wait on them)."""
    n = 0
    for bb in nc.m.functions[0].blocks:
        for inst in bb.instructions:
            if inst.engine != mybir.EngineType.DVE:
                continue
            si = inst.sync_info
            if si is None or not si.on_wait:
                continue
            keep = [w for w in si.on_wait
                    if not str(getattr(w, "ant_name", "")).startswith("DVE")]
            if len(keep) != len(si.on_wait):
                n += len(si.on_wait) - len(keep)
                inst.sync_info = mybir.SyncInfo(
                    on_wait=keep, on_update=list(si.on_update))
    return n


def build_nc(fixup=True, strip_dve=False):
    nc = bass.Bass("TRN2", target_bir_lowering=False, debug=False)
    x1 = nc.dram_tensor("x1", [B, RH, DM], F32, kind="ExternalInput").ap()
    x2 = nc.dram_tensor("x2", [B, X2R, DM], F32, kind="ExternalInput").ap()
    out = nc.dram_tensor("out", [B, K], F16, kind="ExternalOutput").ap()
    with tile.TileContext(nc) as tc, ExitStack() as ctx:
        _body(ctx, tc, out, x1, x2)
    if strip_dve:
        _strip_dve_serial_waits(nc)
    if fixup:
        _walrus_fixup(nc)
    return nc


def _body(ctx, tc, out, x1, x2):
    nc = tc.nc

    sg = ctx.enter_context(tc.tile_pool(name="sg", bufs=1))
    diag_pool = ctx.enter_context(tc.tile_pool(name="diag", bufs=4))
    ps_t = ctx.enter_context(tc.tile_pool(name="ps_t", bufs=2, space="PSUM"))
    ps_t2 = ctx.enter_context(tc.tile_pool(name="ps_t2", bufs=1, space="PSUM"))
    ps_b = ctx.enter_context(tc.tile_pool(name="ps_b", bufs=2, space="PSUM"))
    stage_pool = ctx.enter_context(tc.tile_pool(name="stage", bufs=1, space="DRAM"))

    identF = sg.tile([128, 128], BF16)
    make_identity(nc, identF)
    identF4 = sg.tile([128, 4, 128], BF16)
    for j in range(4):
        nc.gpsimd.tensor_copy(identF4[:, j, :], identF)
    zfill = sg.tile([16, 64], F16)
    nc.gpsimd.memset(zfill, 0.0)

    # natural-layout input staging (f32) and bf16 casts, per 128-row class;
    # classes are consumed within their wave so the pools cycle buffers.
    natp = ctx.enter_context(tc.tile_pool(name="natp", bufs=3))
    natbp = ctx.enter_context(tc.tile_pool(name="natbp", bufs=3))
    natc2p = ctx.enter_context(tc.tile_pool(name="natc2p", bufs=1))
    nat_x1c0 = natp.tile([128, B, DM], F32, name="nat")
    nat_x2c0 = natp.tile([128, B, DM], F32, name="nat")
    natb_x1c0 = natbp.tile([128, B, DM], BF16, name="natb")
    natb_x2c0 = natbp.tile([128, B, DM], BF16, name="natb")

    # sum-of-squares -> (after sqrt) norms; class cols: 0=x1c0 1=x2c0 2=x2c1 3=x1c1
    ss = sg.tile([128, 4, B], F32)
    ssb = sg.tile([128, B // 2], F32)
    iv = sg.tile([128, 4, B], F32)    # 1/norm per class (DVE reciprocal)
    ivb = sg.tile([128, B // 2], F32)
    dumpD = sg.tile([128, 4, DM], BF16)
    dumpD32 = sg.tile([128, 4, DM], F32)
    dumpA = sg.tile([128, DM], F32)

    # transposed normalized bf16 operands
    x1T = sg.tile([128, B, 2, RH], BF16)
    x2T = sg.tile([128, B, 2, X2R], BF16)

    # band staging
    evc = [sg.tile([128, 8, WBLK], F16, name=f"evc{k}") for k in range(NBLK)]
    nc.gpsimd.memset(evc[0][:, :, 114:WBLK], 0.0)  # blk0 junk cols -> c=0
    stages = [stage_pool.tile([B, STG], F16, name=f"stage{k}") for k in range(NBLK)]
    nc.sync.dma_start(
        out=bass.AP(tensor=stages[0].tensor, offset=0, ap=[[STG, B], [1, 64]]),
        in_=zfill,
    )

    D = sg.tile([B, K + 1], F16)
    p = sg.tile([B, K], F16)
    nc.gpsimd.memset(D, DBIG)
    nc.gpsimd.memset(p, DBIG)
    nc.gpsimd.memset(p[:, W:W + 1], 0.0)

    # ---- loads (SP ring), c0 classes first -------------------------------
    for q in range(4):
        bs = slice(4 * q, 4 * q + 4)
        nc.sync.dma_start(out=nat_x1c0[:, bs, :],
                          in_=x1[bs, 0:128, :].rearrange("b p d -> p b d"))
        nc.sync.dma_start(out=nat_x2c0[:, bs, :],
                          in_=x2[bs, 0:128, :].rearrange("b p d -> p b d"))
    nat_x2c1 = natp.tile([128, B, DM], F32, name="nat")
    for q in range(4):
        bs = slice(4 * q, 4 * q + 4)
        nc.sync.dma_start(out=nat_x2c1[:, bs, :],
                          in_=x2[bs, 128:256, :].rearrange("b p d -> p b d"))

    def load_x1c1(nat_t):
        # issued on SP after g1 so the critical stage0/g0/g1 DMAs aren't
        # queued behind these transfers on the DMA engines
        for q in range(4):
            bs = slice(4 * q, 4 * q + 4)
            nc.sync.dma_start(out=nat_t[:, bs, :],
                              in_=x1[bs, 128:256, :].rearrange("b p d -> p b d"))

    def load_c2(nat_t):
        for m in range(8):
            nc.sync.dma_start(out=nat_t[0:50, m, :], in_=x2[2 * m, 256:306, :])
            nc.sync.dma_start(out=nat_t[64:114, m, :],
                              in_=x2[2 * m + 1, 256:306, :])

    # ---- helpers ---------------------------------------------------------
    def cast_quad(dst, src):
        """f32 -> bf16 cast of 4 batches' chunks in one ACT op."""
        nc.scalar.activation(out=dst, in_=src, func=ACTF.Copy)

    def act_rsqrt(out_ap, in_ap):
        """iv = 1/sqrt(ss) on ACT via the reciprocal_sqrt table function.
        bass's activation() helper refuses Rsqrt (accuracy caveat); the
        table interp is plenty for this kernel's 2e-2 gate, and it keeps
        every norm op on ACT (no DVE reciprocal in any stream)."""
        eng = nc.scalar
        ins = [eng.lower_ap(in_ap),
               mybir.ImmediateValue(dtype=mybir.dt.float32, value=0.0),
               mybir.ImmediateValue(dtype=mybir.dt.float32, value=1.0),
               mybir.ImmediateValue(dtype=mybir.dt.float32, value=0.0)]
        return eng.add_instruction(mybir.InstActivation(
            name=nc.get_next_instruction_name(), func=ACTF.Rsqrt,
            ins=ins, outs=[eng.lower_ap(out_ap)]))

    def sq_dve_quad(natq, sscols):
        """Sum-of-squares for 4 batches' bf16 chunks in 2 DVE ops."""
        nc.vector.tensor_tensor(dumpD, natq, natq, ALU.mult)
        nc.vector.tensor_reduce(out=sscols, in_=dumpD,
                                axis=mybir.AxisListType.X, op=ALU.add)

    def sq_act(natap, sscol):
        nc.scalar.activation(out=dumpA[0:natap.shape[0], :], in_=natap,
                             func=ACTF.Square, accum_out=sscol)

    def make_diag(ivap, rows=128, base=0):
        """diag(iv) on Pool from a per-partition 1/norm pointer."""
        dg = diag_pool.tile([128, 128], BF16, name="diag")
        rs = slice(base, base + rows)
        nc.gpsimd.tensor_scalar(out=dg[rs, rs], in0=identF[rs, rs],
                                scalar1=ivap, scalar2=None, op0=ALU.mult)
        return dg

    def tpose_quad(xT, qi, cls_nat, col0, ivq):
        """Transpose+normalize batches 4qi..4qi+3 into xT[:, :, :, col0:col0+128].
        All four diag(1/n) blocks are built in ONE Pool op (broadcast mult)."""
        dg4 = diag_pool.tile([128, 4, 128], BF16, name="diag4")
        nc.gpsimd.tensor_tensor(dg4, identF4,
                                ivq.to_broadcast((128, 4, 128)), ALU.mult)
        pst = ps_t.tile([128, 4, 2, 128], F32, name="pst")
        for j in range(4):
            b = 4 * qi + j
            for dh in range(2):
                nc.tensor.matmul(pst[:, j, dh, :],
                                 cls_nat(b)[:, dh * 128:(dh + 1) * 128],
                                 dg4[:, j, :], start=True, stop=True)
        nc.scalar.activation(out=xT[:, 4 * qi:4 * qi + 4, :, col0:col0 + 128],
                             in_=pst, func=ACTF.Copy)

    def tpose_quad_c2(qi, natb2b):
        """c2: batches 4qi..4qi+3 = natb2b cols 2qi,2qi+1, partitions 0:50/64:114."""
        pst = ps_t2.tile([128, 4, 2, 50], F32, name="pstc2")
        for j in range(4):
            m = 2 * qi + j // 2
            base = 64 * (j % 2)
            rs = slice(base, base + 50)
            dg = make_diag(ivb[rs, m:m + 1], rows=50, base=base)
            for dh in range(2):
                nc.tensor.matmul(pst[:, j, dh, :],
                                 natb2b[rs, m, dh * 128:(dh + 1) * 128],
                                 dg[rs, rs], start=True, stop=True)
        nc.scalar.activation(out=x2T[:, 4 * qi:4 * qi + 4, :, 256:306],
                             in_=pst, func=ACTF.Copy)

    def mm_block(blk):
        j0, j1 = _block_jrange(blk)
        wb = j1 - j0
        i0 = blk * RB
        for t in range(3):
            cnt = 3 if t < 2 else 2
            psb = ps_b.tile([128, 3, WBLK], F32, name="psb")
            for s in range(cnt):
                for pg in range(2):
                    b = (3 * t + s) * 2 + pg
                    for kt in range(2):
                        nc.tensor.matmul(
                            psb[pg * 64:(pg + 1) * 64, s, 0:wb],
                            x1T[:, b, kt, i0:i0 + RB],
                            x2T[:, b, kt, j0:j1],
                            start=(kt == 0), stop=(kt == 1))
            nc.scalar.activation(out=evc[blk][:, 3 * t:3 * t + cnt, 0:wb],
                                 in_=psb[:, 0:cnt, 0:wb], func=ACTF.Copy,
                                 scale=-1.0, bias=1.0)

    def stage_block(blk):
        for pg in range(2):
            nc.sync.dma_start(
                out=bass.AP(tensor=stages[blk].tensor,
                            offset=pg * STG + 64,
                            ap=[[WBLK, 64], [2 * STG, 8], [1, WBLK]]),
                in_=evc[blk][pg * 64:(pg + 1) * 64, :, :],
            )

    band_tiles = []

    def gather_block(g):
        blk, r0 = g // 2, (g % 2) * 32
        bt = sg.tile([B, 32 * K], F16, name=f"band{g}")
        src = bass.AP(
            tensor=stages[blk].tensor,
            offset=64 + (-W if blk == 0 else 0) + r0 * (WBLK + 1),
            ap=[[STG, B], [WBLK + 1, 32], [1, K]],
        )
        nc.sync.dma_start(out=bt, in_=src)
        band_tiles.append(bt)

    # ---- wave c0 ---------------------------------------------------------
    # Casts f32->bf16 as loads land: x2c0 rides the otherwise-idle Pool
    # engine, x1c0 rides ACT, so the cast stream isn't serial on one engine.
    for q in range(4):
        bs = slice(4 * q, 4 * q + 4)
        nc.gpsimd.tensor_copy(natb_x2c0[:, bs, :], nat_x2c0[:, bs, :])
        cast_quad(natb_x1c0[:, bs, :], nat_x1c0[:, bs, :])
    for q in range(4):
        bs = slice(4 * q, 4 * q + 4)
        sq_dve_quad(natb_x2c0[:, bs, :], ss[:, 1, bs])
        sq_dve_quad(natb_x1c0[:, bs, :], ss[:, 0, bs])
        act_rsqrt(iv[:, 0:2, bs], ss[:, 0:2, bs])
    for qi in range(4):
        tpose_quad(x2T, qi, lambda b: natb_x2c0[:, b, :], 0,
                   iv[:, 1, 4 * qi:4 * qi + 4])
        tpose_quad(x1T, qi, lambda b: natb_x1c0[:, b, :], 0,
                   iv[:, 0, 4 * qi:4 * qi + 4])
    mm_block(0)
    stage_block(0)
    gather_block(0)
    gather_block(1)

    # ---- late waves (no DVE ops anywhere in the waves) -------------------
    def wave_full(cls_col, nat_t, natb_t, xT, col0):
        for q in range(4):
            bs = slice(4 * q, 4 * q + 4)
            for b in range(4 * q, 4 * q + 4):
                sq_act(nat_t[:, b, :], ss[:, cls_col, b:b + 1])
            act_rsqrt(iv[:, cls_col, bs], ss[:, cls_col, bs])
            cast_quad(natb_t[:, bs, :], nat_t[:, bs, :])
        for qi in range(4):
            tpose_quad(xT, qi, lambda b: natb_t[:, b, :], col0,
                       iv[:, cls_col, 4 * qi:4 * qi + 4])

    # late loads issue on SP after g1 (keeps DMA engines clear for stage0/g0)
    nat_x1c1 = natp.tile([128, B, DM], F32, name="nat")
    load_x1c1(nat_x1c1)
    nat_c2 = natc2p.tile([128, B // 2, DM], F32, name="natc2")
    load_c2(nat_c2)

    # x2c1
    natb_x2c1 = natbp.tile([128, B, DM], BF16, name="natb")
    wave_full(2, nat_x2c1, natb_x2c1, x2T, 128)
    mm_block(1)
    stage_block(1)
    gather_block(2)
    gather_block(3)

    # x1c1
    natb_x1c1 = natbp.tile([128, B, DM], BF16, name="natb")
    wave_full(3, nat_x1c1, natb_x1c1, x1T, 128)
    mm_block(2)
    stage_block(2)
    gather_block(4)
    gather_block(5)

    # x2c2 (50 rows, pair-packed)
    natb_c2 = natc2p.tile([128, B // 2, DM], BF16, name="natbc2")
    for q in range(4):
        ms = slice(2 * q, 2 * q + 2)
        for m in range(2 * q, 2 * q + 2):
            sq_act(nat_c2[:, m, :], ssb[:, m:m + 1])
        act_rsqrt(ivb[:, ms], ssb[:, ms])
    for m in range(0, 8, 4):
        cast_quad(natb_c2[:, m:m + 4, :], nat_c2[:, m:m + 4, :])
    for qi in range(4):
        tpose_quad_c2(qi, natb_c2)
    mm_block(3)
    stage_block(3)
    gather_block(6)
    gather_block(7)

    # ---- DP --------------------------------------------------------------
    for i in range(DP_ROWS):
        g, r = divmod(i, 32)
        bt = band_tiles[g]
        if i > 0:
            nc.vector.tensor_tensor(p, D[:, 0:K], D[:, 1:K + 1], ALU.min)
        nc.vector.tensor_tensor_scan(
            out=D[:, 0:K], data0=p, data1=bt[:, r * K:(r + 1) * K],
            initial=float(DBIG), op0=ALU.min, op1=ALU.add,
        )

    nc.sync.dma_start(out=out, in_=D[:, 0:K])


def _get_nc():
    global _CACHED_NC
    if _CACHED_NC is None:
        _CACHED_NC = build_nc()
    return _CACHED_NC


def make_in_maps(x1, x2):
    x1 = np.asarray(x1, dtype=np.float32)
    x2 = np.asarray(x2, dtype=np.float32)
    in_maps = []
    for g in range(4):
        sl = slice(g * B, (g + 1) * B)
        in_maps.append({
            "x1": np.ascontiguousarray(x1[sl, 0:RH]),
            "x2": np.ascontiguousarray(x2[sl, 0:X2R]),
        })
    for g in range(4):
        sl = slice(g * B, (g + 1) * B)
        in_maps.append({
            "x1": np.ascontiguousarray(x1[sl, ::-1][:, 0:RH]),
            "x2": np.ascontiguousarray(x2[sl, ::-1][:, 0:X2R]),
        })
    return in_maps


def combine(fwd, bwd):
    """fwd, bwd: [B, 101] boundary rows (any float dtype) -> [B, 1] scores."""
    fwd = np.asarray(fwd, dtype=np.float32)
    bwd = np.asarray(bwd, dtype=np.float32)
    Bpad = np.concatenate([bwd, np.full((bwd.shape[0], 1), BIG, np.float32)], axis=1)
    rev1 = Bpad[:, ::-1][:, 0:K]      # B'[101-k]
    rev2 = bwd[:, ::-1]               # B'[100-k]
    sc = (fwd + np.minimum(rev1, rev2)).min(axis=1)
    return sc.astype(np.float32)[:, None]


def run_spmd(x1, x2, trace=False, **kwargs):
    nc = _get_nc()
    in_maps = make_in_maps(x1, x2)
    res = run_bass_kernel_spmd(nc, in_maps, core_ids=list(range(NCORES)),
                               trace=trace, **kwargs)
    outs = []
    for g in range(4):
        outs.append(combine(res.results[g]["out"], res.results[g + 4]["out"]))
    return np.concatenate(outs, axis=0), res


def kernel(x1, x2):
    outp, _ = run_spmd(x1, x2)
    return outp


# revision 66
# speedup vs baseline: 1.0189x; 1.0189x over previous
"""Banded-DTW (cosine cost, Sakoe-Chiba W=50) Trainium2 Bass kernel, v3.

Forward/backward split across 8 cores as in v2 (core c: fwd rows 0..255 of
batch group c; core c+4: same program on time-reversed inputs). Host combines
boundary rows.

v3 engine plan (per core):
- DVE runs the DP chain in fp16 (tensor_tensor min 2x_1p + tensor_tensor_scan
  4x_2p) plus the c0 sum-of-squares (TT mult + tensor_reduce) in its pre-DP
  idle window.
- Normalization is folded into the PE: each 128-row chunk is transposed via a
  matmul against diag(1/n) (built on Pool via tensor_scalar divide of an
  identity), so no elementwise normalize pass exists anywhere.
- ACT does the remaining squares (Square+accum), quad-batched sqrts, the
  PSUM->SBUF transpose evacuations (bf16, pair-packed), and the band
  evacuations (1 - ps, fp16, 6 batches per PSUM bank via partition-offset
  matmul outputs).
- Cost band staged to DRAM in fp16, diagonal-gathered back as [16, 32*101]
  tiles (same math as v2).
"""

import numpy as np
from contextlib import ExitStack

import concourse.bass as bass
import concourse.tile as tile
from concourse import mybir
from concourse.bass_utils import run_bass_kernel_spmd
from concourse.masks import make_identity

F32 = mybir.dt.float32
BF16 = mybir.dt.bfloat16
F16 = mybir.dt.float16
ALU = mybir.AluOpType
ACTF = mybir.ActivationFunctionType

NCORES = 8
B = 16         # batches per core
RH = 256       # DP rows per core (half of 512)
T = 512
DM = 256
W = 50
K = 2 * W + 1  # 101 band columns
BIG = 1.0e9    # host-side combine 'infinity'
DBIG = 30000.0  # device 'infinity' (fp16-safe)
EPS = 1e-8
X2R = 306      # x2 rows needed per half (j < 306)
RB = 64        # rows per block
NBLK = RH // RB  # 4
WBLK = 164     # staging row stride = max block j-width
STG = 64 + RB * WBLK + 64  # per-batch staging floats for one block
DP_ROWS = RH

USE_DIVIDE = True   # diag = I / n via AluOpType.divide (else reciprocal+mult)

_CACHED_NC = None


def _block_jrange(blk):
    j0 = max(0, blk * RB - W)
    j1 = blk * RB + RB - 1 + W + 1
    return j0, j1  # (0,114) (14,178) (78,242) (142,306)


def _walrus_fixup(nc, max_waits=1):
    """Walrus in this container accepts at most one sync-wait per instruction
    and rejects EVENT_SEMAPHORE_RANGE_CLEAR InstISA; hoist extra waits onto
    standalone InstEventSemaphore waits and drop the range-clear."""
    k = 0
    for bb in nc.m.functions[0].blocks:
        out = []
        for inst in bb.instructions:
            if (type(inst).__name__ == "InstISA"
                    and getattr(inst, "op_name", None) == "EVENT_SEMAPHORE_RANGE_CLEAR"):
                continue
            si = inst.sync_info
            if si is not None and len(si.on_wait) > max_waits:
                waits = list(si.on_wait)
                for w in waits[:-max_waits]:
                    es = mybir.InstEventSemaphore(
                        name=f"eswait_{k}", engine=inst.engine, ins=[], outs=[])
                    es.sync_info = mybir.SyncInfo(on_wait=[w], on_update=[])
                    out.append(es)
                    k += 1
                inst.sync_info = mybir.SyncInfo(
                    on_wait=waits[-max_waits:], on_update=list(si.on_update))
            out.append(inst)
        bb.instructions = out


def _strip_dve_serial_waits(nc):
    """Remove DVE-instruction waits on the DVE engine's own serialization
    semaphore; program order on the in-order DVE queue already sequences
    them. Cross-engine waits (DMA/ACT/Pool sems) are preserved, as are the
    updates (other engines still wait on them)."""
    n = 0
    for bb in nc.m.functions[0].blocks:
        for inst in bb.instructions:
            if inst.engine != mybir.EngineType.DVE:
                continue
            si = inst.sync_info
            if si is None or not si.on_wait:
                continue
            keep = [w for w in si.on_wait
                    if not str(getattr(w, "ant_name", "")).startswith("DVE")]
            if len(keep) != len(si.on_wait):
                n += len(si.on_wait) - len(keep)
                inst.sync_info = mybir.SyncInfo(
                    on_wait=keep, on_update=list(si.on_update))
    return n


def build_nc(fixup=True, strip_dve=False):
    nc = bass.Bass("TRN2", target_bir_lowering=False, debug=False)
    x1 = nc.dram_tensor("x1", [B, RH, DM], F32, kind="ExternalInput").ap()
    x2 = nc.dram_tensor("x2", [B, X2R, DM], F32, kind="ExternalInput").ap()
    out = nc.dram_tensor("out", [B, K], F16, kind="ExternalOutput").ap()
    with tile.TileContext(nc) as tc, ExitStack() as ctx:
        _body(ctx, tc, out, x1, x2)
    if strip_dve:
        _strip_dve_serial_waits(nc)
    if fixup:
        _walrus_fixup(nc)
    return nc


def _body(ctx, tc, out, x1, x2):
    nc = tc.nc

    sg = ctx.enter_context(tc.tile_pool(name="sg", bufs=1))
    diag_pool = ctx.enter_context(tc.tile_pool(name="diag", bufs=4))
    ps_t = ctx.enter_context(tc.tile_pool(name="ps_t", bufs=2, space="PSUM"))
    ps_t2 = ctx.enter_context(tc.tile_pool(name="ps_t2", bufs=1, space="PSUM"))
    ps_b = ctx.enter_context(tc.tile_pool(name="ps_b", bufs=2, space="PSUM"))
    stage_pool = ctx.enter_context(tc.tile_pool(name="stage", bufs=1, space="DRAM"))

    identF = sg.tile([128, 128], BF16)
    make_identity(nc, identF)
    identF4 = sg.tile([128, 4, 128], BF16)
    for j in range(4):
        nc.gpsimd.tensor_copy(identF4[:, j, :], identF)
    zfill = sg.tile([16, 64], F16)
    nc.gpsimd.memset(zfill, 0.0)

    # natural-layout input staging (f32) and bf16 casts, per 128-row class;
    # classes are consumed within their wave so the pools cycle buffers.
    natp = ctx.enter_context(tc.tile_pool(name="natp", bufs=3))
    natbp = ctx.enter_context(tc.tile_pool(name="natbp", bufs=3))
    natc2p = ctx.enter_context(tc.tile_pool(name="natc2p", bufs=1))
    nat_x1c0 = natp.tile([128, B, DM], F32, name="nat")
    nat_x2c0 = natp.tile([128, B, DM], F32, name="nat")
    natb_x1c0 = natbp.tile([128, B, DM], BF16, name="natb")
    natb_x2c0 = natbp.tile([128, B, DM], BF16, name="natb")

    # sum-of-squares -> (after sqrt) norms; class cols: 0=x1c0 1=x2c0 2=x2c1 3=x1c1
    ss = sg.tile([128, 4, B], F32)
    ssb = sg.tile([128, B // 2], F32)
    iv = sg.tile([128, 4, B], F32)    # 1/norm per class (DVE reciprocal)
    ivb = sg.tile([128, B // 2], F32)
    dumpD = sg.tile([128, 4, DM], BF16)
    dumpD32 = sg.tile([128, 4, DM], F32)
    dumpA = sg.tile([128, DM], F32)

    # transposed normalized bf16 operands
    x1T = sg.tile([128, B, 2, RH], BF16)
    x2T = sg.tile([128, B, 2, X2R], BF16)

    # band staging
    evc = [sg.tile([128, 8, WBLK], F16, name=f"evc{k}") for k in range(NBLK)]
    nc.gpsimd.memset(evc[0][:, :, 114:WBLK], 0.0)  # blk0 junk cols -> c=0
    stages = [stage_pool.tile([B, STG], F16, name=f"stage{k}") for k in range(NBLK)]
    nc.sync.dma_start(
        out=bass.AP(tensor=stages[0].tensor, offset=0, ap=[[STG, B], [1, 64]]),
        in_=zfill,
    )

    D = sg.tile([B, K + 1], F16)
    p = sg.tile([B, K], F16)
    nc.gpsimd.memset(D, DBIG)
    nc.gpsimd.memset(p, DBIG)
    nc.gpsimd.memset(p[:, W:W + 1], 0.0)

    # ---- loads (SP ring), c0 classes first -------------------------------
    for q in range(4):
        bs = slice(4 * q, 4 * q + 4)
        nc.sync.dma_start(out=nat_x1c0[:, bs, :],
                          in_=x1[bs, 0:128, :].rearrange("b p d -> p b d"))
        nc.sync.dma_start(out=nat_x2c0[:, bs, :],
                          in_=x2[bs, 0:128, :].rearrange("b p d -> p b d"))
    nat_x2c1 = natp.tile([128, B, DM], F32, name="nat")
    for q in range(4):
        bs = slice(4 * q, 4 * q + 4)
        nc.sync.dma_start(out=nat_x2c1[:, bs, :],
                          in_=x2[bs, 128:256, :].rearrange("b p d -> p b d"))

    def load_x1c1(nat_t):
        # issued on SP after g1 so the critical stage0/g0/g1 DMAs aren't
        # queued behind these transfers on the DMA engines
        for q in range(4):
            bs = slice(4 * q, 4 * q + 4)
            nc.sync.dma_start(out=nat_t[:, bs, :],
                              in_=x1[bs, 128:256, :].rearrange("b p d -> p b d"))

    def load_c2(nat_t):
        for m in range(8):
            nc.sync.dma_start(out=nat_t[0:50, m, :], in_=x2[2 * m, 256:306, :])
            nc.sync.dma_start(out=nat_t[64:114, m, :],
                              in_=x2[2 * m + 1, 256:306, :])

    # ---- helpers ---------------------------------------------------------
    def cast_quad(dst, src):
        """f32 -> bf16 cast of 4 batches' chunks in one ACT op."""
        nc.scalar.activation(out=dst, in_=src, func=ACTF.Copy)

    def act_rsqrt(out_ap, in_ap):
        """iv = 1/sqrt(ss) on ACT via the reciprocal_sqrt table function.
        bass's activation() helper refuses Rsqrt (accuracy caveat); the
        table interp is plenty for this kernel's 2e-2 gate, and it keeps
        every norm op on ACT (no DVE reciprocal in any stream)."""
        eng = nc.scalar
        ins = [eng.lower_ap(in_ap),
               mybir.ImmediateValue(dtype=mybir.dt.float32, value=0.0),
               mybir.ImmediateValue(dtype=mybir.dt.float32, value=1.0),
               mybir.ImmediateValue(dtype=mybir.dt.float32, value=0.0)]
        return eng.add_instruction(mybir.InstActivation(
            name=nc.get_next_instruction_name(), func=ACTF.Rsqrt,
            ins=ins, outs=[eng.lower_ap(out_ap)]))

    def sq_dve_quad(natq, sscols):
        """Sum-of-squares for 4 batches' bf16 chunks in 2 DVE ops."""
        nc.vector.tensor_tensor(dumpD, natq, natq, ALU.mult)
        nc.vector.tensor_reduce(out=sscols, in_=dumpD,
                                axis=mybir.AxisListType.X, op=ALU.add)

    def sq_act(natap, sscol):
        nc.scalar.activation(out=dumpA[0:natap.shape[0], :], in_=natap,
                             func=ACTF.Square, accum_out=sscol)

    def make_diag(ivap, rows=128, base=0):
        """diag(iv) on Pool from a per-partition 1/norm pointer."""
        dg = diag_pool.tile([128, 128], BF16, name="diag")
        rs = slice(base, base + rows)
        nc.gpsimd.tensor_scalar(out=dg[rs, rs], in0=identF[rs, rs],
                                scalar1=ivap, scalar2=None, op0=ALU.mult)
        return dg

    def tpose_quad(xT, qi, cls_nat, col0, ivq):
        """Transpose+normalize batches 4qi..4qi+3 into xT[:, :, :, col0:col0+128].
        All four diag(1/n) blocks are built in ONE Pool op (broadcast mult)."""
        dg4 = diag_pool.tile([128, 4, 128], BF16, name="diag4")
        nc.gpsimd.tensor_tensor(dg4, identF4,
                                ivq.to_broadcast((128, 4, 128)), ALU.mult)
        pst = ps_t.tile([128, 4, 2, 128], F32, name="pst")
        for j in range(4):
            b = 4 * qi + j
            for dh in range(2):
                nc.tensor.matmul(pst[:, j, dh, :],
                                 cls_nat(b)[:, dh * 128:(dh + 1) * 128],
                                 dg4[:, j, :], start=True, stop=True)
        nc.scalar.activation(out=xT[:, 4 * qi:4 * qi + 4, :, col0:col0 + 128],
                             in_=pst, func=ACTF.Copy)

    def tpose_quad_c2(qi, natb2b):
        """c2: batches 4qi..4qi+3 = natb2b cols 2qi,2qi+1, partitions 0:50/64:114."""
        pst = ps_t2.tile([128, 4, 2, 50], F32, name="pstc2")
        for j in range(4):
            m = 2 * qi + j // 2
            base = 64 * (j % 2)
            rs = slice(base, base + 50)
            dg = make_diag(ivb[rs, m:m + 1], rows=50, base=base)
            for dh in range(2):
                nc.tensor.matmul(pst[:, j, dh, :],
                                 natb2b[rs, m, dh * 128:(dh + 1) * 128],
                                 dg[rs, rs], start=True, stop=True)
        nc.scalar.activation(out=x2T[:, 4 * qi:4 * qi + 4, :, 256:306],
                             in_=pst, func=ACTF.Copy)

    def mm_block(blk):
        j0, j1 = _block_jrange(blk)
        wb = j1 - j0
        i0 = blk * RB
        for t in range(3):
            cnt = 3 if t < 2 else 2
            psb = ps_b.tile([128, 3, WBLK], F32, name="psb")
            for s in range(cnt):
                for pg in range(2):
                    b = (3 * t + s) * 2 + pg
                    for kt in range(2):
                        nc.tensor.matmul(
                            psb[pg * 64:(pg + 1) * 64, s, 0:wb],
                            x1T[:, b, kt, i0:i0 + RB],
                            x2T[:, b, kt, j0:j1],
                            start=(kt == 0), stop=(kt == 1))
            nc.scalar.activation(out=evc[blk][:, 3 * t:3 * t + cnt, 0:wb],
                                 in_=psb[:, 0:cnt, 0:wb], func=ACTF.Copy,
                                 scale=-1.0, bias=1.0)

    def stage_block(blk):
        # per-(bank,pg) staging: each trio of batches stages as soon as its
        # own band evacuation lands instead of waiting for all three banks
        for t in range(3):
            s0, cnt = 3 * t, (3 if t < 2 else 2)
            for pg in range(2):
                nc.sync.dma_start(
                    out=bass.AP(tensor=stages[blk].tensor,
                                offset=pg * STG + (2 * s0) * STG + 64,
                                ap=[[WBLK, 64], [2 * STG, cnt], [1, WBLK]]),
                    in_=evc[blk][pg * 64:(pg + 1) * 64, s0:s0 + cnt, :],
                )

    band_tiles = []

    def gather_block(g):
        blk, r0 = g // 2, (g % 2) * 32
        bt = sg.tile([B, 32 * K], F16, name=f"band{g}")
        src = bass.AP(
            tensor=stages[blk].tensor,
            offset=64 + (-W if blk == 0 else 0) + r0 * (WBLK + 1),
            ap=[[STG, B], [WBLK + 1, 32], [1, K]],
        )
        nc.sync.dma_start(out=bt, in_=src)
        band_tiles.append(bt)

    # ---- wave c0 ---------------------------------------------------------
    # Casts f32->bf16 as loads land: x2c0 rides the otherwise-idle Pool
    # engine, x1c0 rides ACT, so the cast stream isn't serial on one engine.
    for q in range(4):
        bs = slice(4 * q, 4 * q + 4)
        nc.gpsimd.tensor_copy(natb_x2c0[:, bs, :], nat_x2c0[:, bs, :])
        cast_quad(natb_x1c0[:, bs, :], nat_x1c0[:, bs, :])
    for q in range(4):
        bs = slice(4 * q, 4 * q + 4)
        sq_dve_quad(natb_x2c0[:, bs, :], ss[:, 1, bs])
        sq_dve_quad(natb_x1c0[:, bs, :], ss[:, 0, bs])
        act_rsqrt(iv[:, 0:2, bs], ss[:, 0:2, bs])
    for qi in range(4):
        tpose_quad(x2T, qi, lambda b: natb_x2c0[:, b, :], 0,
                   iv[:, 1, 4 * qi:4 * qi + 4])
        tpose_quad(x1T, qi, lambda b: natb_x1c0[:, b, :], 0,
                   iv[:, 0, 4 * qi:4 * qi + 4])
    mm_block(0)
    stage_block(0)
    gather_block(0)
    gather_block(1)

    # ---- late waves (no DVE ops anywhere in the waves) -------------------
    def wave_full(cls_col, nat_t, natb_t, xT, col0):
        for q in range(4):
            bs = slice(4 * q, 4 * q + 4)
            for b in range(4 * q, 4 * q + 4):
                sq_act(nat_t[:, b, :], ss[:, cls_col, b:b + 1])
            act_rsqrt(iv[:, cls_col, bs], ss[:, cls_col, bs])
            cast_quad(natb_t[:, bs, :], nat_t[:, bs, :])
        for qi in range(4):
            tpose_quad(xT, qi, lambda b: natb_t[:, b, :], col0,
                       iv[:, cls_col, 4 * qi:4 * qi + 4])

    # late loads issue on SP after g1 (keeps DMA engines clear for stage0/g0)
    nat_x1c1 = natp.tile([128, B, DM], F32, name="nat")
    load_x1c1(nat_x1c1)
    nat_c2 = natc2p.tile([128, B // 2, DM], F32, name="natc2")
    load_c2(nat_c2)

    # x2c1
    natb_x2c1 = natbp.tile([128, B, DM], BF16, name="natb")
    wave_full(2, nat_x2c1, natb_x2c1, x2T, 128)
    mm_block(1)
    stage_block(1)
    gather_block(2)
    gather_block(3)

    # x1c1
    natb_x1c1 = natbp.tile([128, B, DM], BF16, name="natb")
    wave_full(3, nat_x1c1, natb_x1c1, x1T, 128)
    mm_block(2)
    stage_block(2)
    gather_block(4)
    gather_block(5)

    # x2c2 (50 rows, pair-packed)
    natb_c2 = natc2p.tile([128, B // 2, DM], BF16, name="natbc2")
    for q in range(4):
        ms = slice(2 * q, 2 * q + 2)
        for m in range(2 * q, 2 * q + 2):
            sq_act(nat_c2[:, m, :], ssb[:, m:m + 1])
        act_rsqrt(ivb[:, ms], ssb[:, ms])
    for m in range(0, 8, 4):
        cast_quad(natb_c2[:, m:m + 4, :], nat_c2[:, m:m + 4, :])
    for qi in range(4):
        tpose_quad_c2(qi, natb_c2)
    mm_block(3)
    stage_block(3)
    gather_block(6)
    gather_block(7)

    # ---- DP --------------------------------------------------------------
    # Rows i < W have no valid band cells below k = W-i; those cells stay at
    # the initial BIG forever (the covered range grows monotonically), so the
    # min and scan shrink to [W-i, K) exactly - shorter ops, same values.
    for i in range(DP_ROWS):
        g, r = divmod(i, 32)
        bt = band_tiles[g]
        sc = max(0, W - i)
        if i > 0:
            nc.vector.tensor_tensor(p[:, sc:K], D[:, sc:K], D[:, sc + 1:K + 1],
                                    ALU.min)
        nc.vector.tensor_tensor_scan(
            out=D[:, sc:K], data0=p[:, sc:K],
            data1=bt[:, r * K + sc:(r + 1) * K],
            initial=float(DBIG), op0=ALU.min, op1=ALU.add,
        )

    nc.sync.dma_start(out=out, in_=D[:, 0:K])


def _get_nc():
    global _CACHED_NC
    if _CACHED_NC is None:
        _CACHED_NC = build_nc()
    return _CACHED_NC


def make_in_maps(x1, x2):
    x1 = np.asarray(x1, dtype=np.float32)
    x2 = np.asarray(x2, dtype=np.float32)
    in_maps = []
    for g in range(4):
        sl = slice(g * B, (g + 1) * B)
        in_maps.append({
            "x1": np.ascontiguousarray(x1[sl, 0:RH]),
            "x2": np.ascontiguousarray(x2[sl, 0:X2R]),
        })
    for g in range(4):
        sl = slice(g * B, (g + 1) * B)
        in_maps.append({
            "x1": np.ascontiguousarray(x1[sl, ::-1][:, 0:RH]),
            "x2": np.ascontiguousarray(x2[sl, ::-1][:, 0:X2R]),
        })
    return in_maps


def combine(fwd, bwd):
    """fwd, bwd: [B, 101] boundary rows (any float dtype) -> [B, 1] scores."""
    fwd = np.asarray(fwd, dtype=np.float32)
    bwd = np.asarray(bwd, dtype=np.float32)
    Bpad = np.concatenate([bwd, np.full((bwd.shape[0], 1), BIG, np.float32)], axis=1)
    rev1 = Bpad[:, ::-1][:, 0:K]      # B'[101-k]
    rev2 = bwd[:, ::-1]               # B'[100-k]
    sc = (fwd + np.minimum(rev1, rev2)).min(axis=1)
    return sc.astype(np.float32)[:, None]


def run_spmd(x1, x2, trace=False, **kwargs):
    nc = _get_nc()
    in_maps = make_in_maps(x1, x2)
    res = run_bass_kernel_spmd(nc, in_maps, core_ids=list(range(NCORES)),
                               trace=trace, **kwargs)
    outs = []
    for g in range(4):
        outs.append(combine(res.results[g]["out"], res.results[g + 4]["out"]))
    return np.concatenate(outs, axis=0), res


def kernel(x1, x2):
    outp, _ = run_spmd(x1, x2)
    return outp
